# revision 31
# baseline (speedup 1.0000x reference)
"""Trainium2 Bass kernel for nn_AmandaModel (QA model: 4 BiGRU layers,
passage-question attention, MultiFactor self-attention, gated FF, span loss).

Strategy
--------
- Data-parallel over batch: 8 batch elements -> 8 NeuronCores, parameters
  replicated; no collectives. Per-core output is the (sb, se) span logits;
  the tiny final -sum(log softmax*softmax) is folded on host from those.
- The 4 sequential BiGRU layers (512 steps each) are NOT run step-by-step
  (per-step instruction overheads would cost ~3ms). Instead each BiGRU is
  solved parallel-in-time with Picard iterations: all T gate pre-activations
  computed as one [128,T] matmul from the shifted trajectory guess, and the
  gated blend h_t = z_t*h_{t-1} + (1-z_t)*n_t applied exactly with the
  hardware `tensor_tensor_scan` (affine prefix scan along the free dim).
  4 iterations reach the bf16 noise floor (validated in numpy/CoreSim/HW).
- Layout: fused directions. Partitions 0:64 = forward hidden, 64:128 =
  backward hidden with time REVERSED, so both directions scan "forward"
  along the free axis in the same instruction.
"""

import os
import sys

sys.path.insert(0, "/opt/trn_rl_repo")

import ml_dtypes
import numpy as np

import concourse.bass as bass
import concourse.bacc as bacc
import concourse.mybir as mybir
import concourse.tile as tile
from concourse.bass_utils import run_bass_kernel_spmd

F32 = mybir.dt.float32
BF16 = mybir.dt.bfloat16
I32 = mybir.dt.int32
AF = mybir.ActivationFunctionType
OP = mybir.AluOpType
AX = mybir.AxisListType

H = 64
HID = 128
EMB = 100
NF = 10
VOCAB = 30000
TP = 512
TQ = 32
B = 8
NCORES = 8

# Picard iteration counts per layer
KITS = {"share": 4, "enc": 4, "begin": 4, "end": 4}

LAST_EXEC_NS = None
LAST_RESULTS = None

_BUILT = None


# --------------------------------------------------------------------------
# graph building helpers
# --------------------------------------------------------------------------

def _bigru_picard(nc, tc, out_pool, name, lw, ident_bf, x_nat, x_rev, T, Tq, n_iter):
    """Parallel-in-time BiGRU solve; returns traj bf16 tile [128, T+Tq+2]:
    col 0 = 0, cols 1..T = main scan, col T+1 = 0, cols T+2..T+Tq+1 = Q scan.

    lw: dict with gil (list of [rows,384] bf16 lhsT K-chunk tiles), gil_rows,
    rec ([128,384] bf16 block-diag recurrent lhsT), brz ([128,2] f32),
    bn ([128,2] f32). x_nat/x_rev: [rows_c, T+Tq] bf16 input tiles per chunk.
    """
    Tt = T + Tq
    gil, gil_rows = lw["gil"], lw["gil_rows"]
    rec, brz, bn = lw["rec"], lw["brz"], lw["bn"]
    nchunks = len(gil)

    traj = out_pool.tile([128, Tt + 2], BF16, name=f"{name}_traj")
    nc.gpsimd.memset(traj[:], 0.0)

    with tc.tile_pool(name=f"{name}_sb", bufs=1) as sp:
        gi_t = [sp.tile([128, Tt], BF16, name=f"{name}_gi{g}") for g in range(3)]

        # ---- gi precompute: gi_g = Wih x + bias, both directions ----
        with tc.tile_pool(name=f"{name}_gips", bufs=2, space="PSUM") as gp:
            for g in range(3):
                ps_p = gp.tile([128, T], F32, tag="gip", name=f"{name}_gip{g}")
                ps_q = (gp.tile([128, Tq], F32, tag="giq",
                                name=f"{name}_giq{g}") if Tq else None)
                # each PSUM half must finish its accumulation group before
                # the other half's group opens (shared zero region)
                for c in range(nchunks):
                    rows, lt = gil_rows[c], gil[c]
                    nc.tensor.matmul(ps_p[0:64, :], lhsT=lt[0:rows, 128 * g:128 * g + 64],
                                     rhs=x_nat[c][0:rows, 0:T], start=c == 0,
                                     stop=c == nchunks - 1)
                for c in range(nchunks):
                    rows, lt = gil_rows[c], gil[c]
                    nc.tensor.matmul(ps_p[64:128, :], lhsT=lt[0:rows, 128 * g + 64:128 * g + 128],
                                     rhs=x_rev[c][0:rows, 0:T], start=c == 0,
                                     stop=c == nchunks - 1)
                if Tq:
                    for c in range(nchunks):
                        rows, lt = gil_rows[c], gil[c]
                        nc.tensor.matmul(ps_q[0:64, :], lhsT=lt[0:rows, 128 * g:128 * g + 64],
                                         rhs=x_nat[c][0:rows, T:Tt], start=c == 0,
                                         stop=c == nchunks - 1)
                    for c in range(nchunks):
                        rows, lt = gil_rows[c], gil[c]
                        nc.tensor.matmul(ps_q[64:128, :], lhsT=lt[0:rows, 128 * g + 64:128 * g + 128],
                                         rhs=x_rev[c][0:rows, T:Tt], start=c == 0,
                                         stop=c == nchunks - 1)
                bias = brz[:, g:g + 1] if g < 2 else bn[:, 0:1]
                nc.scalar.activation(gi_t[g][:, 0:T], ps_p[:], AF.Identity, bias=bias)
                if Tq:
                    nc.scalar.activation(gi_t[g][:, T:Tt], ps_q[:], AF.Identity, bias=bias)

        # ---- Picard iterations ----
        r_bf = sp.tile([128, Tt], BF16, name=f"{name}_r")
        z_bf = sp.tile([128, Tt], BF16, name=f"{name}_z")
        omz = sp.tile([128, Tt], BF16, name=f"{name}_omz")
        t1 = sp.tile([128, Tt], BF16, name=f"{name}_t1")
        an = sp.tile([128, Tt], BF16, name=f"{name}_an")
        n_bf = sp.tile([128, Tt], BF16, name=f"{name}_n")
        d1 = sp.tile([128, Tt], BF16, name=f"{name}_d1")

        with tc.tile_pool(name=f"{name}_itps", bufs=1, space="PSUM") as ip:
            # PSUM dependency tracking is bank-granular, so each column half
            # of each gate gets its OWN bank tile: the half-1 gate matmuls
            # then truly depend only on the previous iteration's half-1 scan,
            # shrinking the serial Picard cycle to half-width ops. The three
            # question segments share one extra bank (sequential groups).
            T2 = T // 2
            psh = [[ip.tile([128, T2], F32, name=f"{name}_ps{g}h{h}")
                    for h in range(2)] for g in range(3)]
            psq = ip.tile([128, 3 * Tq], F32, name=f"{name}_psq") if Tq else None

            for _it in range(n_iter):
                # per (half, gate) accumulation groups; gi accumulate opens
                # (independent of traj), gate matmul closes. r first (its
                # sigmoid gates the chain), then n (stt needs it), z last.
                for h, (a, b) in enumerate(((0, T2), (T2, T))):
                    for g in (0, 2, 1):
                        if g != 2:
                            nc.tensor.matmul(psh[g][h][:], lhsT=ident_bf[:],
                                             rhs=gi_t[g][:, a:b], start=True, stop=False)
                        nc.tensor.matmul(psh[g][h][:], lhsT=rec[:, 128 * g:128 * (g + 1)],
                                         rhs=traj[:, a:b], start=(g == 2), stop=True)
                if Tq:
                    for g in (0, 2, 1):
                        if g != 2:
                            nc.tensor.matmul(psq[:, Tq * g:Tq * (g + 1)], lhsT=ident_bf[:],
                                             rhs=gi_t[g][:, T:Tt], start=True, stop=False)
                        nc.tensor.matmul(psq[:, Tq * g:Tq * (g + 1)],
                                         lhsT=rec[:, 128 * g:128 * (g + 1)],
                                         rhs=traj[:, T + 1:Tt + 1], start=(g == 2), stop=True)

                # elementwise chain in column halves aligned with the PSUM
                # banks; the question segment is its own small tail
                segs = [(0, T2, 0), (T2, T, 1), (T, Tt, None)] if Tq else \
                       [(0, T2, 0), (T2, T, 1)]
                for (a, b, h) in segs:
                    src = [psh[g][h][:] if h is not None
                           else psq[:, Tq * g:Tq * (g + 1)] for g in range(3)]
                    nc.scalar.activation(r_bf[:, a:b], src[0], AF.Sigmoid)
                    nc.scalar.activation(z_bf[:, a:b], src[1], AF.Sigmoid)
                    nc.vector.scalar_tensor_tensor(out=t1[:, a:b], in0=src[2],
                                                   scalar=bn[:, 1:2], in1=r_bf[:, a:b],
                                                   op0=OP.add, op1=OP.mult)
                    nc.vector.tensor_tensor(out=an[:, a:b], in0=t1[:, a:b],
                                            in1=gi_t[2][:, a:b], op=OP.add)
                    nc.scalar.activation(n_bf[:, a:b], an[:, a:b], AF.Tanh)
                    # omz = 1 - z (gpsimd: off the critical DVE path)
                    nc.gpsimd.tensor_scalar(out=omz[:, a:b], in0=z_bf[:, a:b],
                                            scalar1=-1.0, scalar2=1.0,
                                            op0=OP.mult, op1=OP.add)
                    nc.vector.tensor_tensor(out=d1[:, a:b], in0=omz[:, a:b],
                                            in1=n_bf[:, a:b], op=OP.mult)
                # h = scan: state = z*state + d1 (halves chained via initial)
                nc.vector.tensor_tensor_scan(out=traj[:, 1:T2 + 1], data0=z_bf[:, 0:T2],
                                             data1=d1[:, 0:T2], initial=0.0,
                                             op0=OP.mult, op1=OP.add)
                nc.vector.tensor_tensor_scan(out=traj[:, T2 + 1:T + 1],
                                             data0=z_bf[:, T2:T], data1=d1[:, T2:T],
                                             initial=traj[:, T2:T2 + 1],
                                             op0=OP.mult, op1=OP.add)
                if Tq:
                    nc.vector.tensor_tensor_scan(out=traj[:, T + 2:Tt + 2],
                                                 data0=z_bf[:, T:Tt], data1=d1[:, T:Tt],
                                                 initial=0.0, op0=OP.mult, op1=OP.add)
    return traj


def _mats_from_traj(nc, pool, name, traj, T, off, rev=True):
    """Model-layout matrices from a scan trajectory: mat[:, t] = [h_f[t]; h_b[t]]
    (time-natural); matrev time-reversed. off = traj column of scan position 0."""
    mat = pool.tile([128, T], BF16, name=f"{name}_mat")
    nc.vector.tensor_copy(out=mat[0:64, :], in_=traj[0:64, off:off + T])
    nc.vector.tensor_copy(out=mat[64:128, :], in_=traj[64:128, off:off + T][:, ::-1])
    if not rev:
        return mat, None
    matr = pool.tile([128, T], BF16, name=f"{name}_matrev")
    nc.gpsimd.tensor_copy(out=matr[0:64, :], in_=traj[0:64, off:off + T][:, ::-1])
    nc.gpsimd.tensor_copy(out=matr[64:128, :], in_=traj[64:128, off:off + T])
    return mat, matr


def _build_graph():
    nc = bacc.Bacc()

    # ---- DRAM parameters ----
    emb_d = nc.declare_dram_parameter("emb", [VOCAB, EMB], F32, isOutput=False)
    idx_d = nc.declare_dram_parameter("idx", [128, 5], I32, isOutput=False)
    lay_d = {}
    for lname, D in (("share", EMB), ("enc", 2 * HID), ("begin", 2 * HID), ("end", HID)):
        lay_d[lname] = dict(
            gil=nc.declare_dram_parameter(f"{lname}_gil", [D, 384], BF16, isOutput=False),
            rec=nc.declare_dram_parameter(f"{lname}_rec", [128, 384], BF16, isOutput=False),
            brz=nc.declare_dram_parameter(f"{lname}_brz", [128, 2], F32, isOutput=False),
            bn=nc.declare_dram_parameter(f"{lname}_bn", [128, 2], F32, isOutput=False),
            D=D,
        )
    wg_d = nc.declare_dram_parameter("wg", [2 * HID, 2 * HID], BF16, isOutput=False)
    bg_d = nc.declare_dram_parameter("bg", [128, 2], F32, isOutput=False)
    wq_d = nc.declare_dram_parameter("wq", [3 * HID, HID], BF16, isOutput=False)
    bq_d = nc.declare_dram_parameter("bq", [128, 1], F32, isOutput=False)
    ctxm_d = nc.declare_dram_parameter("ctxm", [NF * HID, HID], BF16, isOutput=False)
    identb_d = nc.declare_dram_parameter("identb", [128, 128], BF16, isOutput=False)
    identf_d = nc.declare_dram_parameter("identf", [128, 128], F32, isOutput=False)
    out_d = nc.declare_dram_parameter("out", [2, TP], F32, isOutput=True)

    Tt = TP + TQ

    with tile.TileContext(nc) as tc, \
         tc.tile_pool(name="weights", bufs=1) as wp, \
         tc.tile_pool(name="acts", bufs=1) as ap:

        # ---- load weights to SBUF ----
        idx_sb = wp.tile([128, 5], I32, name="idx_sb")
        nc.sync.dma_start(out=idx_sb[:], in_=idx_d[:])
        ident_bf = wp.tile([128, 128], BF16, name="ident_bf")
        nc.sync.dma_start(out=ident_bf[:], in_=identb_d[:])
        ident_f = wp.tile([128, 128], F32, name="ident_f")
        nc.sync.dma_start(out=ident_f[:], in_=identf_d[:])

        lay = {}
        for lname in ("share", "enc", "begin", "end"):
            D = lay_d[lname]["D"]
            nch = (D + 127) // 128
            gil, gil_rows = [], []
            for c in range(nch):
                rows = min(128, D - 128 * c)
                t = wp.tile([rows, 384], BF16, name=f"{lname}_gil{c}")
                nc.sync.dma_start(out=t[:], in_=lay_d[lname]["gil"][128 * c:128 * c + rows, :])
                gil.append(t)
                gil_rows.append(rows)
            rec = wp.tile([128, 384], BF16, name=f"{lname}_rec")
            nc.sync.dma_start(out=rec[:], in_=lay_d[lname]["rec"][:])
            brz = wp.tile([128, 2], F32, name=f"{lname}_brz")
            nc.sync.dma_start(out=brz[:], in_=lay_d[lname]["brz"][:])
            bn = wp.tile([128, 2], F32, name=f"{lname}_bn")
            nc.sync.dma_start(out=bn[:], in_=lay_d[lname]["bn"][:])
            lay[lname] = dict(gil=gil, gil_rows=gil_rows, rec=rec, brz=brz, bn=bn)

        wg = []
        for c in range(2):
            t = wp.tile([128, 2 * HID], BF16, name=f"wg_sb{c}")
            nc.sync.dma_start(out=t[:], in_=wg_d[128 * c:128 * (c + 1), :])
            wg.append(t)
        bg = wp.tile([128, 2], F32, name="bg_sb")
        nc.sync.dma_start(out=bg[:], in_=bg_d[:])
        wq = []
        for c in range(3):
            t = wp.tile([128, HID], BF16, name=f"wq_sb{c}")
            nc.sync.dma_start(out=t[:], in_=wq_d[128 * c:128 * (c + 1), :])
            wq.append(t)
        bq = wp.tile([128, 1], F32, name="bq_sb")
        nc.sync.dma_start(out=bq[:], in_=bq_d[:])
        ctxm = []
        for f in range(NF):
            t = wp.tile([128, HID], BF16, name=f"ctxm_sb{f}")
            nc.sync.dma_start(out=t[:], in_=ctxm_d[128 * f:128 * (f + 1), :])
            ctxm.append(t)

        # =========================================================
        # embedding gather + transpose into [EMB, 544] layout
        # =========================================================
        x_nat = ap.tile([EMB, Tt], BF16, name="x_nat")
        x_rev = ap.tile([EMB, Tt], BF16, name="x_rev")
        with tc.tile_pool(name="embg", bufs=2) as eg, \
             tc.tile_pool(name="embps", bufs=2, space="PSUM") as eps:
            for j in range(5):
                g = eg.tile([128, EMB], F32, tag="gath", name=f"gath{j}")
                nc.gpsimd.indirect_dma_start(
                    out=g[:], out_offset=None, in_=emb_d[:],
                    in_offset=bass.IndirectOffsetOnAxis(ap=idx_sb[:, j:j + 1], axis=0))
                pst = eps.tile([EMB, 128], F32, tag="embt", name=f"embt{j}")
                nc.tensor.transpose(pst[:], g[:], ident_f[:])
                if j < 4:
                    nc.scalar.activation(x_nat[:, 128 * j:128 * (j + 1)], pst[:], AF.Copy)
                    nc.vector.tensor_copy(
                        out=x_rev[:, TP - 128 * (j + 1):TP - 128 * j][:, ::-1], in_=pst[:])
                else:
                    nc.scalar.activation(x_nat[:, TP:Tt], pst[:, 0:TQ], AF.Copy)
                    nc.vector.tensor_copy(out=x_rev[:, TP:Tt][:, ::-1], in_=pst[:, 0:TQ])

        # =========================================================
        # share BiGRU (passage + question fused)
        # =========================================================
        straj = _bigru_picard(nc, tc, ap, "share", lay["share"], ident_bf,
                              [x_nat], [x_rev], TP, TQ, KITS["share"])
        Pmat, Prev = _mats_from_traj(nc, ap, "P", straj, TP, 1)
        Qmat, _ = _mats_from_traj(nc, ap, "Q", straj, TQ, TP + 2, rev=False)

        # =========================================================
        # passage-question attention
        # =========================================================
        with tc.tile_pool(name="attps", bufs=1, space="PSUM") as aps:
            ps_att = aps.tile([128, 128], F32, name="ps_att")
            for c in range(4):
                nc.tensor.matmul(ps_att[:, 32 * c:32 * (c + 1)],
                                 lhsT=Pmat[:, 128 * c:128 * (c + 1)], rhs=Qmat[:],
                                 start=True, stop=True)
            # row softmax over q (per 32-col chunk)
            rmax = ap.tile([128, 4], F32, name="rmax")
            nc.vector.tensor_reduce(out=rmax[:], in_=ps_att[:].rearrange("p (c q) -> p c q", q=32),
                                    axis=AX.X, op=OP.max)
            nrm = ap.tile([128, 4], F32, name="nrm")
            nc.vector.tensor_scalar(out=nrm[:], in0=rmax[:], scalar1=-1.0, scalar2=None,
                                    op0=OP.mult)
            e_sb = ap.tile([128, 128], F32, name="e_sb")
            for c in range(4):
                nc.scalar.activation(e_sb[:, 32 * c:32 * (c + 1)], ps_att[:, 32 * c:32 * (c + 1)],
                                     AF.Exp, bias=nrm[:, c:c + 1])
            rsum = ap.tile([128, 4], F32, name="rsum")
            nc.vector.tensor_reduce(out=rsum[:], in_=e_sb[:].rearrange("p (c q) -> p c q", q=32),
                                    axis=AX.X, op=OP.add)
            rrec = ap.tile([128, 4], F32, name="rrec")
            nc.vector.reciprocal(out=rrec[:], in_=rsum[:])
            rel = ap.tile([128, 128], BF16, name="rel")
            for c in range(4):
                nc.vector.tensor_scalar(out=rel[:, 32 * c:32 * (c + 1)],
                                        in0=e_sb[:, 32 * c:32 * (c + 1)],
                                        scalar1=rrec[:, c:c + 1], scalar2=None, op0=OP.mult)
            att_sb = ap.tile([128, 128], F32, name="att_sb")
            nc.scalar.activation(att_sb[:], ps_att[:], AF.Copy)

        attT = ap.tile([32, TP], F32, name="attT")
        relT = ap.tile([32, TP], BF16, name="relT")
        QT = ap.tile([32, 128], BF16, name="QT")
        with tc.tile_pool(name="attps2", bufs=2, space="PSUM") as aps2:
            for c in range(4):
                pT = aps2.tile([32, 128], F32, tag="aT", name=f"aT{c}")
                nc.tensor.transpose(pT[:], att_sb[:, 32 * c:32 * (c + 1)], ident_f[:])
                nc.scalar.activation(attT[:, 128 * c:128 * (c + 1)], pT[:], AF.Copy)
                pT2 = aps2.tile([32, 128], BF16, tag="rT", name=f"rT{c}")
                nc.tensor.transpose(pT2[:], rel[:, 32 * c:32 * (c + 1)], ident_bf[:])
                nc.vector.tensor_copy(out=relT[:, 128 * c:128 * (c + 1)], in_=pT2[:])
            pQT = aps2.tile([32, 128], BF16, tag="qT", name="pQT")
            nc.tensor.transpose(pQT[:], Qmat[:], ident_bf[:])
            nc.vector.tensor_copy(out=QT[:], in_=pQT[:])

        maxc = ap.tile([32, 1], F32, name="maxc")
        nc.vector.tensor_reduce(out=maxc[:], in_=attT[:], axis=AX.X, op=OP.max)
        maxcb = ap.tile([32, 1], BF16, name="maxcb")
        nc.vector.tensor_copy(out=maxcb[:], in_=maxc[:])

        qrep = ap.tile([128, 1], BF16, name="qrep")
        quev = ap.tile([128, TP], BF16, name="quev")
        qurev = ap.tile([128, TP], BF16, name="qurev")
        with tc.tile_pool(name="qps", bufs=1, space="PSUM") as qps:
            ps_qma = qps.tile([128, 1], F32, name="ps_qma")
            nc.tensor.matmul(ps_qma[:], lhsT=QT[:], rhs=maxcb[:], start=True, stop=True)
            qma = ap.tile([128, 1], BF16, name="qma")
            nc.scalar.activation(qma[:], ps_qma[:], AF.Copy)
            # qrep = tanh(Wq^T q + bq), q = [qma; Q0; Q1]
            ps_qr = qps.tile([128, 1], F32, name="ps_qr")
            nc.tensor.matmul(ps_qr[:], lhsT=wq[0][:], rhs=qma[:], start=True, stop=False)
            nc.tensor.matmul(ps_qr[:], lhsT=wq[1][:], rhs=Qmat[:, 0:1], start=False, stop=False)
            nc.tensor.matmul(ps_qr[:], lhsT=wq[2][:], rhs=Qmat[:, 1:2], start=False, stop=True)
            nc.scalar.activation(qrep[:], ps_qr[:], AF.Tanh, bias=bq[:])

            # quev^T [128(d), 512(p)]
            ps_quev = qps.tile([128, TP], F32, name="ps_quev")
            nc.tensor.matmul(ps_quev[:], lhsT=QT[:], rhs=relT[:], start=True, stop=True)
            nc.scalar.activation(quev[:], ps_quev[:], AF.Copy)
            nc.gpsimd.tensor_copy(out=qurev[:, ::-1], in_=quev[:])

        # =========================================================
        # enc BiGRU -> V
        # =========================================================
        etraj = _bigru_picard(nc, tc, ap, "enc", lay["enc"], ident_bf,
                              [Pmat, quev], [Prev, qurev], TP, 0, KITS["enc"])
        Vmat, _ = _mats_from_traj(nc, ap, "V", etraj, TP, 1, rev=False)

        # V^T chunks [128(j), 128(d)]
        VT = [ap.tile([128, HID], BF16, name=f"VT{c}") for c in range(4)]
        with tc.tile_pool(name="vtps", bufs=2, space="PSUM") as vps:
            for c in range(4):
                pv = vps.tile([128, 128], BF16, tag="vt", name=f"vt{c}")
                nc.tensor.transpose(pv[:], Vmat[:, 128 * c:128 * (c + 1)], ident_bf[:])
                nc.scalar.activation(VT[c][:], pv[:], AF.Copy)

        # =========================================================
        # MultiFactor: Smax = max_f (V M_f V^T), A = softmax, MF = A V
        # =========================================================
        smax = [ap.tile([128, TP], F32, name=f"smax{c}") for c in range(4)]
        with tc.tile_pool(name="mfps", bufs=2, space="PSUM") as mp, \
             tc.tile_pool(name="mfu", bufs=2) as mu:
            for f in range(NF):
                ps_u = mp.tile([128, TP], F32, tag="u", name=f"u{f}")
                nc.tensor.matmul(ps_u[:], lhsT=ctxm[f][:], rhs=Vmat[:],
                                 start=True, stop=True)
                u_sb = mu.tile([128, TP], BF16, tag="usb", name=f"usb{f}")
                nc.scalar.activation(u_sb[:], ps_u[:], AF.Copy)
                for c in range(4):
                    ps_s = mp.tile([128, TP], F32, tag="s", name=f"s{f}_{c}")
                    nc.tensor.matmul(ps_s[:], lhsT=u_sb[:, 128 * c:128 * (c + 1)], rhs=Vmat[:],
                                     start=True, stop=True)
                    if f == 0:
                        nc.scalar.activation(smax[c][:], ps_s[:], AF.Copy)
                    else:
                        nc.vector.tensor_tensor(out=smax[c][:], in0=ps_s[:],
                                                in1=smax[c][:], op=OP.max)

        # softmax rows of smax -> A chunks
        Achunks = []
        for c in range(4):
            am = ap.tile([128, 1], F32, name=f"amx{c}")
            nc.vector.tensor_reduce(out=am[:], in_=smax[c][:], axis=AX.X, op=OP.max)
            amn = ap.tile([128, 1], F32, name=f"amn{c}")
            nc.vector.tensor_scalar(out=amn[:], in0=am[:], scalar1=-1.0, scalar2=None,
                                    op0=OP.mult)
            ae = ap.tile([128, TP], F32, name=f"ae{c}")
            nc.scalar.activation(ae[:], smax[c][:], AF.Exp, bias=amn[:])
            asum = ap.tile([128, 1], F32, name=f"asum{c}")
            nc.vector.tensor_reduce(out=asum[:], in_=ae[:], axis=AX.X, op=OP.add)
            arec = ap.tile([128, 1], F32, name=f"arec{c}")
            nc.vector.reciprocal(out=arec[:], in_=asum[:])
            a_c = ap.tile([128, TP], BF16, name=f"ac{c}")
            eng = nc.vector if c % 2 == 0 else nc.gpsimd
            eng.tensor_scalar(out=a_c[:], in0=ae[:], scalar1=arec[:], scalar2=None,
                              op0=OP.mult)
            Achunks.append(a_c)

        # A^T assembled per j-chunk: ATj[j] [128(j), 512(p)]
        ATj = [ap.tile([128, TP], BF16, name=f"atj{j}") for j in range(4)]
        with tc.tile_pool(name="atps", bufs=3, space="PSUM") as atp:
            for c in range(4):
                for j in range(4):
                    pt = atp.tile([128, 128], BF16, tag="at", name=f"at{c}_{j}")
                    nc.tensor.transpose(pt[:], Achunks[c][:, 128 * j:128 * (j + 1)], ident_bf[:])
                    if (c + j) % 2 == 0:
                        nc.vector.tensor_copy(out=ATj[j][:, 128 * c:128 * (c + 1)],
                                              in_=pt[:])
                    else:
                        nc.scalar.activation(ATj[j][:, 128 * c:128 * (c + 1)],
                                             pt[:], AF.Copy)

        # MF^T [128(d), 512(p)] then gated FF -> Y^T chunks
        MFT = ap.tile([128, TP], BF16, name="MFT")
        Ych, Yrev = [], []
        with tc.tile_pool(name="mftps", bufs=2, space="PSUM") as mfp:
            ps_mft = mfp.tile([128, TP], F32, name="ps_mft")
            for j in range(4):
                nc.tensor.matmul(ps_mft[:], lhsT=VT[j][:], rhs=ATj[j][:],
                                 start=(j == 0), stop=(j == 3))
            nc.scalar.activation(MFT[:], ps_mft[:], AF.Copy)

            for cc in range(2):
                ps_g = mfp.tile([128, TP], F32, tag="gff", name=f"gff{cc}")
                nc.tensor.matmul(ps_g[:], lhsT=wg[0][:, 128 * cc:128 * (cc + 1)], rhs=MFT[:],
                                 start=True, stop=False)
                nc.tensor.matmul(ps_g[:], lhsT=wg[1][:, 128 * cc:128 * (cc + 1)], rhs=Vmat[:],
                                 start=False, stop=True)
                sg = ap.tile([128, TP], BF16, name=f"sg{cc}")
                nc.scalar.activation(sg[:], ps_g[:], AF.Sigmoid, bias=bg[:, cc:cc + 1])
                y = ap.tile([128, TP], BF16, name=f"y{cc}")
                src = MFT if cc == 0 else Vmat
                nc.vector.tensor_tensor(out=y[:], in0=src[:], in1=sg[:], op=OP.mult)
                yr = ap.tile([128, TP], BF16, name=f"yr{cc}")
                nc.gpsimd.tensor_copy(out=yr[:, ::-1], in_=y[:])
                Ych.append(y)
                Yrev.append(yr)

        # =========================================================
        # begin / end BiGRUs, span logits
        # =========================================================
        btraj = _bigru_picard(nc, tc, ap, "begin", lay["begin"], ident_bf,
                              Ych, Yrev, TP, 0, KITS["begin"])
        Bmat, Brev = _mats_from_traj(nc, ap, "B", btraj, TP, 1)

        etraj2 = _bigru_picard(nc, tc, ap, "end", lay["end"], ident_bf,
                               [Bmat], [Brev], TP, 0, KITS["end"])
        Emat, _ = _mats_from_traj(nc, ap, "E", etraj2, TP, 1, rev=False)

        with tc.tile_pool(name="outps", bufs=1, space="PSUM") as op_:
            ps_sb = op_.tile([1, TP], F32, name="ps_sb")
            nc.tensor.matmul(ps_sb[:], lhsT=qrep[:], rhs=Bmat[:], start=True, stop=True)
            sb_sb = ap.tile([1, TP], F32, name="sb_sb")
            nc.scalar.activation(sb_sb[:], ps_sb[:], AF.Copy)
            ps_se = op_.tile([1, TP], F32, name="ps_se")
            nc.tensor.matmul(ps_se[:], lhsT=qrep[:], rhs=Emat[:], start=True, stop=True)
            se_sb = ap.tile([1, TP], F32, name="se_sb")
            nc.scalar.activation(se_sb[:], ps_se[:], AF.Copy)
            nc.sync.dma_start(out=out_d[0:1, :], in_=sb_sb[:])
            nc.sync.dma_start(out=out_d[1:2, :], in_=se_sb[:])

    return nc


# --------------------------------------------------------------------------
# host-side weight packing
# --------------------------------------------------------------------------

def _pack_layer(pf, pb):
    """Build gil [D,384], rec [128,384], brz [128,2], bn [128,2] from the
    (Wih, Whh, bih, bhh) tuples of both directions."""
    Wih_f, Whh_f, bih_f, bhh_f = [np.asarray(a, np.float32) for a in pf]
    Wih_b, Whh_b, bih_b, bhh_b = [np.asarray(a, np.float32) for a in pb]
    D = Wih_f.shape[1]
    gil = np.zeros((D, 384), np.float32)
    rec = np.zeros((128, 384), np.float32)
    for g in range(3):
        gil[:, 128 * g:128 * g + 64] = Wih_f[64 * g:64 * (g + 1), :].T
        gil[:, 128 * g + 64:128 * g + 128] = Wih_b[64 * g:64 * (g + 1), :].T
        rec[0:64, 128 * g:128 * g + 64] = Whh_f[64 * g:64 * (g + 1), :].T
        rec[64:128, 128 * g + 64:128 * g + 128] = Whh_b[64 * g:64 * (g + 1), :].T
    brz = np.zeros((128, 2), np.float32)
    bn = np.zeros((128, 2), np.float32)
    for g in range(2):
        brz[0:64, g] = bih_f[64 * g:64 * (g + 1)] + bhh_f[64 * g:64 * (g + 1)]
        brz[64:128, g] = bih_b[64 * g:64 * (g + 1)] + bhh_b[64 * g:64 * (g + 1)]
    bn[0:64, 0] = bih_f[128:192]
    bn[64:128, 0] = bih_b[128:192]
    bn[0:64, 1] = bhh_f[128:192]
    bn[64:128, 1] = bhh_b[128:192]
    bf = ml_dtypes.bfloat16
    return gil.astype(bf), rec.astype(bf), brz, bn


def _get_built():
    global _BUILT
    if _BUILT is None:
        _BUILT = _build_graph()
        _BUILT.finalize()
    return _BUILT


def _build_noop():
    """Minimal graph for calibrating the axon RPC floor."""
    nc = bacc.Bacc()
    i_d = nc.declare_dram_parameter("nin", [2, TP], F32, isOutput=False)
    o_d = nc.declare_dram_parameter("out", [2, TP], F32, isOutput=True)
    with tile.TileContext(nc) as tc, tc.tile_pool(name="p", bufs=1) as pp:
        t = pp.tile([2, TP], F32, name="t")
        nc.sync.dma_start(out=t[:], in_=i_d[:])
        nc.sync.dma_start(out=o_d[:], in_=t[:])
    return nc


class _Runner:
    """Persistent jit executor mirroring bass2jax.run_bass_via_pjrt's
    multi-core path, with resident inputs for repeat timing."""

    def __init__(self, nc):
        if not nc.is_finalized():
            nc.finalize()
        import jax
        from jax.experimental.shard_map import shard_map
        from jax.sharding import Mesh, PartitionSpec
        from concourse import bass2jax as b2j

        b2j.install_neuronx_cc_hook()
        self.nc = nc
        pname = nc.partition_id_tensor.name if nc.partition_id_tensor else None
        in_names, out_names, out_avals, zero_outs = [], [], [], []
        for alloc in nc.m.functions[0].allocations:
            if not isinstance(alloc, mybir.MemoryLocationSet):
                continue
            name = alloc.memorylocations[0].name
            if alloc.kind == "ExternalInput":
                if name != pname:
                    in_names.append(name)
            elif alloc.kind == "ExternalOutput":
                out_names.append(name)
                shape = tuple(alloc.tensor_shape)
                dtype = mybir.dt.np(alloc.dtype)
                out_avals.append(jax.core.ShapedArray(shape, dtype))
                zero_outs.append(np.zeros(shape, dtype))
        self.in_names, self.out_names = in_names, out_names
        self.out_avals, self.zero_outs = out_avals, zero_outs
        n_params, n_outs = len(in_names), len(out_names)
        all_names = in_names + out_names
        if pname is not None:
            all_names = all_names + [pname]

        def _body(*args):
            operands = list(args)
            if pname is not None:
                operands.append(b2j.partition_id_tensor())
            outs = b2j._bass_exec_p.bind(
                *operands, out_avals=tuple(out_avals), in_names=tuple(all_names),
                out_names=tuple(out_names), lowering_input_output_aliases=(),
                sim_require_finite=True, sim_require_nnan=True, nc=nc)
            return tuple(outs)

        devices = jax.devices()[:NCORES]
        self.mesh = Mesh(np.asarray(devices), ("core",))
        in_specs = (PartitionSpec("core"),) * (n_params + n_outs)
        out_specs = (PartitionSpec("core"),) * n_outs
        self.fn = jax.jit(
            shard_map(_body, mesh=self.mesh, in_specs=in_specs,
                      out_specs=out_specs, check_rep=False),
            donate_argnums=tuple(range(n_params, n_params + n_outs)),
            keep_unused=True)
        self._resident = None

    def set_inputs(self, in_maps):
        import jax
        from jax.sharding import NamedSharding, PartitionSpec

        sh = NamedSharding(self.mesh, PartitionSpec("core"))
        self._resident = [
            jax.device_put(
                np.concatenate([np.asarray(m[name]) for m in in_maps], axis=0), sh)
            for name in self.in_names
        ]

    def run(self):
        import jax
        from jax.sharding import NamedSharding, PartitionSpec

        sh = NamedSharding(self.mesh, PartitionSpec("core"))
        zeros = [jax.device_put(
            np.zeros((NCORES * z.shape[0], *z.shape[1:]), z.dtype), sh)
            for z in self.zero_outs]
        outs = self.fn(*self._resident, *zeros)
        outs = [np.asarray(o) for o in outs]
        return [
            {name: outs[i].reshape(NCORES, *self.out_avals[i].shape)[c]
             for i, name in enumerate(self.out_names)}
            for c in range(NCORES)
        ]

    def time_ns(self, iters=12):
        import time as _t
        best = float("inf")
        for _ in range(iters):
            t0 = _t.perf_counter()
            self.run()
            dt = _t.perf_counter() - t0
            best = min(best, dt)
        return best * 1e9


_RUNNER = None
_NOOP_NS = None


def kernel(passages, questions, params):
    global LAST_EXEC_NS, LAST_RESULTS, _RUNNER, _NOOP_NS
    passages = np.asarray(passages)
    questions = np.asarray(questions)
    p = dict(params)
    bf = ml_dtypes.bfloat16

    common = {
        "emb": np.asarray(p["emb"], np.float32),
        "wg": np.asarray(p["Wg"], np.float32).astype(bf),
        "wq": np.asarray(p["Wq"], np.float32).astype(bf),
        "bq": np.asarray(p["bq"], np.float32).reshape(128, 1),
        "ctxm": np.asarray(p["ctx_M"], np.float32).reshape(NF * HID, HID).astype(bf),
        "bg": np.asarray(p["bg"], np.float32).reshape(2, 128).T.copy(),
        "identb": np.eye(128, dtype=np.float32).astype(bf),
        "identf": np.eye(128, dtype=np.float32),
    }
    for lname in ("share", "enc", "begin", "end"):
        gil, rec, brz, bn = _pack_layer(p[f"{lname}_f"], p[f"{lname}_b"])
        common[f"{lname}_gil"] = gil
        common[f"{lname}_rec"] = rec
        common[f"{lname}_brz"] = brz
        common[f"{lname}_bn"] = bn

    in_maps = []
    for b_ in range(B):
        toks = np.concatenate([np.asarray(passages[b_], np.int64),
                               np.asarray(questions[b_], np.int64),
                               np.zeros(96, np.int64)])
        idx = toks.reshape(5, 128).T.astype(np.int32).copy()
        m = dict(common)
        m["idx"] = idx
        in_maps.append(m)

    nc = _get_built()
    if _RUNNER is None:
        _RUNNER = _Runner(nc)
    _RUNNER.set_inputs(in_maps)
    results = _RUNNER.run()
    LAST_RESULTS = results

    if bool(int(os.environ.get("BASS_PROFILE", "0"))):
        full_ns = _RUNNER.time_ns()
        if _NOOP_NS is None:
            nr = _Runner(_build_noop())
            nr.set_inputs([{"nin": np.zeros((2, TP), np.float32)} for _ in range(NCORES)])
            nr.run()
            _NOOP_NS = nr.time_ns()
        LAST_EXEC_NS = full_ns - _NOOP_NS
        print(f"[timing] full={full_ns/1e3:.1f}us noop={_NOOP_NS/1e3:.1f}us "
              f"-> kernel ~{LAST_EXEC_NS/1e3:.1f}us")

    loss = 0.0
    for b_ in range(B):
        o = np.asarray(results[b_]["out"], np.float64)
        sb, se = o[0], o[1]
        for v in (sb, se):
            mx = v.max()
            loss += TP * (mx + np.log(np.exp(v - mx).sum()))
        loss -= sb.sum() + se.sum()
    return np.float32(loss)


# revision 36
# speedup vs baseline: 6.1721x; 6.1721x over previous
"""Trainium2 Bass kernel for nn_AmandaModel (QA model: 4 BiGRU layers,
passage-question attention, MultiFactor self-attention, gated FF, span loss).

Strategy
--------
- Data-parallel over batch: 8 batch elements -> 8 NeuronCores, parameters
  replicated; no collectives. Per-core output is the (sb, se) span logits;
  the tiny final -sum(log softmax*softmax) is folded on host from those.
- The 4 sequential BiGRU layers (512 steps each) are NOT run step-by-step
  (per-step instruction overheads would cost ~3ms). Instead each BiGRU is
  solved parallel-in-time with Picard iterations: all T gate pre-activations
  computed as one [128,T] matmul from the shifted trajectory guess, and the
  gated blend h_t = z_t*h_{t-1} + (1-z_t)*n_t applied exactly with the
  hardware `tensor_tensor_scan` (affine prefix scan along the free dim).
  4 iterations reach the bf16 noise floor (validated in numpy/CoreSim/HW).
- Layout: fused directions. Partitions 0:64 = forward hidden, 64:128 =
  backward hidden with time REVERSED, so both directions scan "forward"
  along the free axis in the same instruction.
"""

import os
import sys

sys.path.insert(0, "/opt/trn_rl_repo")

import ml_dtypes
import numpy as np

import concourse.bass as bass
import concourse.bacc as bacc
import concourse.mybir as mybir
import concourse.tile as tile
from concourse.bass_utils import run_bass_kernel_spmd

F32 = mybir.dt.float32
BF16 = mybir.dt.bfloat16
I32 = mybir.dt.int32
AF = mybir.ActivationFunctionType
OP = mybir.AluOpType
AX = mybir.AxisListType

H = 64
HID = 128
EMB = 100
NF = 10
VOCAB = 30000
TP = 512
TQ = 32
B = 8
NCORES = 8

# Picard iteration counts per layer
KITS = {"share": 4, "enc": 4, "begin": 4, "end": 4}

LAST_EXEC_NS = None
LAST_RESULTS = None

_BUILT = None


# --------------------------------------------------------------------------
# graph building helpers
# --------------------------------------------------------------------------

def _bigru_picard(nc, tc, out_pool, name, lw, ident_bf, x_nat, x_rev, T, Tq, n_iter):
    """Parallel-in-time BiGRU solve; returns traj bf16 tile [128, T+Tq+2]:
    col 0 = 0, cols 1..T = main scan, col T+1 = 0, cols T+2..T+Tq+1 = Q scan.

    lw: dict with gil (list of [rows,384] bf16 lhsT K-chunk tiles), gil_rows,
    rec ([128,384] bf16 block-diag recurrent lhsT), brz ([128,2] f32),
    bn ([128,2] f32). x_nat/x_rev: [rows_c, T+Tq] bf16 input tiles per chunk.
    """
    Tt = T + Tq
    gil, gil_rows = lw["gil"], lw["gil_rows"]
    rec, brz, bn = lw["rec"], lw["brz"], lw["bn"]
    nchunks = len(gil)

    traj = out_pool.tile([128, Tt + 2], BF16, name=f"{name}_traj")
    nc.gpsimd.memset(traj[:], 0.0)

    with tc.tile_pool(name=f"{name}_sb", bufs=1) as sp:
        gi_t = [sp.tile([128, Tt], BF16, name=f"{name}_gi{g}") for g in range(3)]

        # ---- gi precompute: gi_g = Wih x + bias, both directions ----
        with tc.tile_pool(name=f"{name}_gips", bufs=2, space="PSUM") as gp:
            for g in range(3):
                ps_p = gp.tile([128, T], F32, tag="gip", name=f"{name}_gip{g}")
                ps_q = (gp.tile([128, Tq], F32, tag="giq",
                                name=f"{name}_giq{g}") if Tq else None)
                # each PSUM half must finish its accumulation group before
                # the other half's group opens (shared zero region)
                for c in range(nchunks):
                    rows, lt = gil_rows[c], gil[c]
                    nc.tensor.matmul(ps_p[0:64, :], lhsT=lt[0:rows, 128 * g:128 * g + 64],
                                     rhs=x_nat[c][0:rows, 0:T], start=c == 0,
                                     stop=c == nchunks - 1)
                for c in range(nchunks):
                    rows, lt = gil_rows[c], gil[c]
                    nc.tensor.matmul(ps_p[64:128, :], lhsT=lt[0:rows, 128 * g + 64:128 * g + 128],
                                     rhs=x_rev[c][0:rows, 0:T], start=c == 0,
                                     stop=c == nchunks - 1)
                if Tq:
                    for c in range(nchunks):
                        rows, lt = gil_rows[c], gil[c]
                        nc.tensor.matmul(ps_q[0:64, :], lhsT=lt[0:rows, 128 * g:128 * g + 64],
                                         rhs=x_nat[c][0:rows, T:Tt], start=c == 0,
                                         stop=c == nchunks - 1)
                    for c in range(nchunks):
                        rows, lt = gil_rows[c], gil[c]
                        nc.tensor.matmul(ps_q[64:128, :], lhsT=lt[0:rows, 128 * g + 64:128 * g + 128],
                                         rhs=x_rev[c][0:rows, T:Tt], start=c == 0,
                                         stop=c == nchunks - 1)
                bias = brz[:, g:g + 1] if g < 2 else bn[:, 0:1]
                nc.scalar.activation(gi_t[g][:, 0:T], ps_p[:], AF.Identity, bias=bias)
                if Tq:
                    nc.scalar.activation(gi_t[g][:, T:Tt], ps_q[:], AF.Identity, bias=bias)

        # ---- Picard iterations ----
        r_bf = sp.tile([128, Tt], BF16, name=f"{name}_r")
        z_bf = sp.tile([128, Tt], BF16, name=f"{name}_z")
        omz = sp.tile([128, Tt], BF16, name=f"{name}_omz")
        t1 = sp.tile([128, Tt], BF16, name=f"{name}_t1")
        an = sp.tile([128, Tt], BF16, name=f"{name}_an")
        n_bf = sp.tile([128, Tt], BF16, name=f"{name}_n")
        d1 = sp.tile([128, Tt], BF16, name=f"{name}_d1")

        with tc.tile_pool(name=f"{name}_itps", bufs=1, space="PSUM") as ip:
            # PSUM dependency tracking is bank-granular, so each column half
            # of each gate gets its OWN bank tile: the half-1 gate matmuls
            # then truly depend only on the previous iteration's half-1 scan,
            # shrinking the serial Picard cycle to half-width ops. The three
            # question segments share one extra bank (sequential groups).
            T2 = T // 2
            psh = [[ip.tile([128, T2], F32, name=f"{name}_ps{g}h{h}")
                    for h in range(2)] for g in range(3)]
            psq = ip.tile([128, 3 * Tq], F32, name=f"{name}_psq") if Tq else None

            for _it in range(n_iter):
                # per (half, gate) accumulation groups; gi accumulate opens
                # (independent of traj), gate matmul closes. r first (its
                # sigmoid gates the chain), then n (stt needs it), z last.
                for h, (a, b) in enumerate(((0, T2), (T2, T))):
                    for g in (0, 2, 1):
                        if g != 2:
                            nc.tensor.matmul(psh[g][h][:], lhsT=ident_bf[:],
                                             rhs=gi_t[g][:, a:b], start=True, stop=False)
                        nc.tensor.matmul(psh[g][h][:], lhsT=rec[:, 128 * g:128 * (g + 1)],
                                         rhs=traj[:, a:b], start=(g == 2), stop=True)
                if Tq:
                    for g in (0, 2, 1):
                        if g != 2:
                            nc.tensor.matmul(psq[:, Tq * g:Tq * (g + 1)], lhsT=ident_bf[:],
                                             rhs=gi_t[g][:, T:Tt], start=True, stop=False)
                        nc.tensor.matmul(psq[:, Tq * g:Tq * (g + 1)],
                                         lhsT=rec[:, 128 * g:128 * (g + 1)],
                                         rhs=traj[:, T + 1:Tt + 1], start=(g == 2), stop=True)

                # elementwise chain in column halves aligned with the PSUM
                # banks; the question segment is its own small tail
                segs = [(0, T2, 0), (T2, T, 1), (T, Tt, None)] if Tq else \
                       [(0, T2, 0), (T2, T, 1)]
                for (a, b, h) in segs:
                    src = [psh[g][h][:] if h is not None
                           else psq[:, Tq * g:Tq * (g + 1)] for g in range(3)]
                    nc.scalar.activation(r_bf[:, a:b], src[0], AF.Sigmoid)
                    nc.scalar.activation(z_bf[:, a:b], src[1], AF.Sigmoid)
                    nc.vector.scalar_tensor_tensor(out=t1[:, a:b], in0=src[2],
                                                   scalar=bn[:, 1:2], in1=r_bf[:, a:b],
                                                   op0=OP.add, op1=OP.mult)
                    nc.vector.tensor_tensor(out=an[:, a:b], in0=t1[:, a:b],
                                            in1=gi_t[2][:, a:b], op=OP.add)
                    nc.scalar.activation(n_bf[:, a:b], an[:, a:b], AF.Tanh)
                    # omz = 1 - z (gpsimd: off the critical DVE path)
                    nc.gpsimd.tensor_scalar(out=omz[:, a:b], in0=z_bf[:, a:b],
                                            scalar1=-1.0, scalar2=1.0,
                                            op0=OP.mult, op1=OP.add)
                    nc.vector.tensor_tensor(out=d1[:, a:b], in0=omz[:, a:b],
                                            in1=n_bf[:, a:b], op=OP.mult)
                # h = scan: state = z*state + d1 (halves chained via initial)
                nc.vector.tensor_tensor_scan(out=traj[:, 1:T2 + 1], data0=z_bf[:, 0:T2],
                                             data1=d1[:, 0:T2], initial=0.0,
                                             op0=OP.mult, op1=OP.add)
                nc.vector.tensor_tensor_scan(out=traj[:, T2 + 1:T + 1],
                                             data0=z_bf[:, T2:T], data1=d1[:, T2:T],
                                             initial=traj[:, T2:T2 + 1],
                                             op0=OP.mult, op1=OP.add)
                if Tq:
                    nc.vector.tensor_tensor_scan(out=traj[:, T + 2:Tt + 2],
                                                 data0=z_bf[:, T:Tt], data1=d1[:, T:Tt],
                                                 initial=0.0, op0=OP.mult, op1=OP.add)
    return traj


def _mats_from_traj(nc, pool, name, traj, T, off, rev=True):
    """Model-layout matrices from a scan trajectory: mat[:, t] = [h_f[t]; h_b[t]]
    (time-natural); matrev time-reversed. off = traj column of scan position 0."""
    mat = pool.tile([128, T], BF16, name=f"{name}_mat")
    nc.vector.tensor_copy(out=mat[0:64, :], in_=traj[0:64, off:off + T])
    nc.vector.tensor_copy(out=mat[64:128, :], in_=traj[64:128, off:off + T][:, ::-1])
    if not rev:
        return mat, None
    matr = pool.tile([128, T], BF16, name=f"{name}_matrev")
    nc.gpsimd.tensor_copy(out=matr[0:64, :], in_=traj[0:64, off:off + T][:, ::-1])
    nc.gpsimd.tensor_copy(out=matr[64:128, :], in_=traj[64:128, off:off + T])
    return mat, matr


def _build_graph():
    nc = bacc.Bacc()

    # ---- DRAM parameters ----
    emb_d = nc.declare_dram_parameter("emb", [VOCAB, EMB], F32, isOutput=False)
    idx_d = nc.declare_dram_parameter("idx", [128, 5], I32, isOutput=False)
    lay_d = {}
    for lname, D in (("share", EMB), ("enc", 2 * HID), ("begin", 2 * HID), ("end", HID)):
        lay_d[lname] = dict(
            gil=nc.declare_dram_parameter(f"{lname}_gil", [D, 384], BF16, isOutput=False),
            rec=nc.declare_dram_parameter(f"{lname}_rec", [128, 384], BF16, isOutput=False),
            brz=nc.declare_dram_parameter(f"{lname}_brz", [128, 2], F32, isOutput=False),
            bn=nc.declare_dram_parameter(f"{lname}_bn", [128, 2], F32, isOutput=False),
            D=D,
        )
    wg_d = nc.declare_dram_parameter("wg", [2 * HID, 2 * HID], BF16, isOutput=False)
    bg_d = nc.declare_dram_parameter("bg", [128, 2], F32, isOutput=False)
    wq_d = nc.declare_dram_parameter("wq", [3 * HID, HID], BF16, isOutput=False)
    bq_d = nc.declare_dram_parameter("bq", [128, 1], F32, isOutput=False)
    ctxm_d = nc.declare_dram_parameter("ctxm", [NF * HID, HID], BF16, isOutput=False)
    identb_d = nc.declare_dram_parameter("identb", [128, 128], BF16, isOutput=False)
    identf_d = nc.declare_dram_parameter("identf", [128, 128], F32, isOutput=False)
    out_d = nc.declare_dram_parameter("out", [2, TP], F32, isOutput=True)

    Tt = TP + TQ

    with tile.TileContext(nc) as tc, \
         tc.tile_pool(name="weights", bufs=1) as wp, \
         tc.tile_pool(name="acts", bufs=1) as ap:

        # ---- load weights to SBUF ----
        idx_sb = wp.tile([128, 5], I32, name="idx_sb")
        nc.sync.dma_start(out=idx_sb[:], in_=idx_d[:])
        ident_bf = wp.tile([128, 128], BF16, name="ident_bf")
        nc.sync.dma_start(out=ident_bf[:], in_=identb_d[:])
        ident_f = wp.tile([128, 128], F32, name="ident_f")
        nc.sync.dma_start(out=ident_f[:], in_=identf_d[:])

        lay = {}
        for lname in ("share", "enc", "begin", "end"):
            dma_eng = nc.sync
            D = lay_d[lname]["D"]
            nch = (D + 127) // 128
            gil, gil_rows = [], []
            for c in range(nch):
                rows = min(128, D - 128 * c)
                t = wp.tile([rows, 384], BF16, name=f"{lname}_gil{c}")
                dma_eng.dma_start(out=t[:], in_=lay_d[lname]["gil"][128 * c:128 * c + rows, :])
                gil.append(t)
                gil_rows.append(rows)
            rec = wp.tile([128, 384], BF16, name=f"{lname}_rec")
            dma_eng.dma_start(out=rec[:], in_=lay_d[lname]["rec"][:])
            brz = wp.tile([128, 2], F32, name=f"{lname}_brz")
            dma_eng.dma_start(out=brz[:], in_=lay_d[lname]["brz"][:])
            bn = wp.tile([128, 2], F32, name=f"{lname}_bn")
            dma_eng.dma_start(out=bn[:], in_=lay_d[lname]["bn"][:])
            lay[lname] = dict(gil=gil, gil_rows=gil_rows, rec=rec, brz=brz, bn=bn)

        wg = []
        for c in range(2):
            t = wp.tile([128, 2 * HID], BF16, name=f"wg_sb{c}")
            nc.sync.dma_start(out=t[:], in_=wg_d[128 * c:128 * (c + 1), :])
            wg.append(t)
        bg = wp.tile([128, 2], F32, name="bg_sb")
        nc.sync.dma_start(out=bg[:], in_=bg_d[:])
        wq = []
        for c in range(3):
            t = wp.tile([128, HID], BF16, name=f"wq_sb{c}")
            nc.sync.dma_start(out=t[:], in_=wq_d[128 * c:128 * (c + 1), :])
            wq.append(t)
        bq = wp.tile([128, 1], F32, name="bq_sb")
        nc.sync.dma_start(out=bq[:], in_=bq_d[:])
        ctxm = []
        for f in range(NF):
            t = wp.tile([128, HID], BF16, name=f"ctxm_sb{f}")
            nc.sync.dma_start(out=t[:], in_=ctxm_d[128 * f:128 * (f + 1), :])
            ctxm.append(t)

        # =========================================================
        # embedding gather + transpose into [EMB, 544] layout
        # =========================================================
        x_nat = ap.tile([EMB, Tt], BF16, name="x_nat")
        x_rev = ap.tile([EMB, Tt], BF16, name="x_rev")
        with tc.tile_pool(name="embg", bufs=2) as eg, \
             tc.tile_pool(name="embps", bufs=2, space="PSUM") as eps:
            for j in range(5):
                g = eg.tile([128, EMB], F32, tag="gath", name=f"gath{j}")
                nc.gpsimd.indirect_dma_start(
                    out=g[:], out_offset=None, in_=emb_d[:],
                    in_offset=bass.IndirectOffsetOnAxis(ap=idx_sb[:, j:j + 1], axis=0))
                pst = eps.tile([EMB, 128], F32, tag="embt", name=f"embt{j}")
                nc.tensor.transpose(pst[:], g[:], ident_f[:])
                if j < 4:
                    nc.scalar.activation(x_nat[:, 128 * j:128 * (j + 1)], pst[:], AF.Copy)
                    nc.vector.tensor_copy(
                        out=x_rev[:, TP - 128 * (j + 1):TP - 128 * j][:, ::-1], in_=pst[:])
                else:
                    nc.scalar.activation(x_nat[:, TP:Tt], pst[:, 0:TQ], AF.Copy)
                    nc.vector.tensor_copy(out=x_rev[:, TP:Tt][:, ::-1], in_=pst[:, 0:TQ])

        # =========================================================
        # share BiGRU (passage + question fused)
        # =========================================================
        straj = _bigru_picard(nc, tc, ap, "share", lay["share"], ident_bf,
                              [x_nat], [x_rev], TP, TQ, KITS["share"])
        Pmat, Prev = _mats_from_traj(nc, ap, "P", straj, TP, 1)
        Qmat, _ = _mats_from_traj(nc, ap, "Q", straj, TQ, TP + 2, rev=False)

        # =========================================================
        # passage-question attention
        # =========================================================
        with tc.tile_pool(name="attps", bufs=1, space="PSUM") as aps:
            ps_att = aps.tile([128, 128], F32, name="ps_att")
            for c in range(4):
                nc.tensor.matmul(ps_att[:, 32 * c:32 * (c + 1)],
                                 lhsT=Pmat[:, 128 * c:128 * (c + 1)], rhs=Qmat[:],
                                 start=True, stop=True)
            # row softmax over q (per 32-col chunk)
            rmax = ap.tile([128, 4], F32, name="rmax")
            nc.vector.tensor_reduce(out=rmax[:], in_=ps_att[:].rearrange("p (c q) -> p c q", q=32),
                                    axis=AX.X, op=OP.max)
            nrm = ap.tile([128, 4], F32, name="nrm")
            nc.vector.tensor_scalar(out=nrm[:], in0=rmax[:], scalar1=-1.0, scalar2=None,
                                    op0=OP.mult)
            e_sb = ap.tile([128, 128], F32, name="e_sb")
            for c in range(4):
                nc.scalar.activation(e_sb[:, 32 * c:32 * (c + 1)], ps_att[:, 32 * c:32 * (c + 1)],
                                     AF.Exp, bias=nrm[:, c:c + 1])
            rsum = ap.tile([128, 4], F32, name="rsum")
            nc.vector.tensor_reduce(out=rsum[:], in_=e_sb[:].rearrange("p (c q) -> p c q", q=32),
                                    axis=AX.X, op=OP.add)
            rrec = ap.tile([128, 4], F32, name="rrec")
            nc.vector.reciprocal(out=rrec[:], in_=rsum[:])
            rel = ap.tile([128, 128], BF16, name="rel")
            for c in range(4):
                nc.vector.tensor_scalar(out=rel[:, 32 * c:32 * (c + 1)],
                                        in0=e_sb[:, 32 * c:32 * (c + 1)],
                                        scalar1=rrec[:, c:c + 1], scalar2=None, op0=OP.mult)
            att_sb = ap.tile([128, 128], F32, name="att_sb")
            nc.scalar.activation(att_sb[:], ps_att[:], AF.Copy)

        attT = ap.tile([32, TP], F32, name="attT")
        relT = ap.tile([32, TP], BF16, name="relT")
        QT = ap.tile([32, 128], BF16, name="QT")
        with tc.tile_pool(name="attps2", bufs=2, space="PSUM") as aps2:
            for c in range(4):
                pT = aps2.tile([32, 128], F32, tag="aT", name=f"aT{c}")
                nc.tensor.transpose(pT[:], att_sb[:, 32 * c:32 * (c + 1)], ident_f[:])
                nc.scalar.activation(attT[:, 128 * c:128 * (c + 1)], pT[:], AF.Copy)
                pT2 = aps2.tile([32, 128], BF16, tag="rT", name=f"rT{c}")
                nc.tensor.transpose(pT2[:], rel[:, 32 * c:32 * (c + 1)], ident_bf[:])
                nc.vector.tensor_copy(out=relT[:, 128 * c:128 * (c + 1)], in_=pT2[:])
            pQT = aps2.tile([32, 128], BF16, tag="qT", name="pQT")
            nc.tensor.transpose(pQT[:], Qmat[:], ident_bf[:])
            nc.vector.tensor_copy(out=QT[:], in_=pQT[:])

        maxc = ap.tile([32, 1], F32, name="maxc")
        nc.vector.tensor_reduce(out=maxc[:], in_=attT[:], axis=AX.X, op=OP.max)
        maxcb = ap.tile([32, 1], BF16, name="maxcb")
        nc.vector.tensor_copy(out=maxcb[:], in_=maxc[:])

        qrep = ap.tile([128, 1], BF16, name="qrep")
        quev = ap.tile([128, TP], BF16, name="quev")
        qurev = ap.tile([128, TP], BF16, name="qurev")
        with tc.tile_pool(name="qps", bufs=1, space="PSUM") as qps:
            ps_qma = qps.tile([128, 1], F32, name="ps_qma")
            nc.tensor.matmul(ps_qma[:], lhsT=QT[:], rhs=maxcb[:], start=True, stop=True)
            qma = ap.tile([128, 1], BF16, name="qma")
            nc.scalar.activation(qma[:], ps_qma[:], AF.Copy)
            # qrep = tanh(Wq^T q + bq), q = [qma; Q0; Q1]
            ps_qr = qps.tile([128, 1], F32, name="ps_qr")
            nc.tensor.matmul(ps_qr[:], lhsT=wq[0][:], rhs=qma[:], start=True, stop=False)
            nc.tensor.matmul(ps_qr[:], lhsT=wq[1][:], rhs=Qmat[:, 0:1], start=False, stop=False)
            nc.tensor.matmul(ps_qr[:], lhsT=wq[2][:], rhs=Qmat[:, 1:2], start=False, stop=True)
            nc.scalar.activation(qrep[:], ps_qr[:], AF.Tanh, bias=bq[:])

            # quev^T [128(d), 512(p)]
            ps_quev = qps.tile([128, TP], F32, name="ps_quev")
            nc.tensor.matmul(ps_quev[:], lhsT=QT[:], rhs=relT[:], start=True, stop=True)
            nc.scalar.activation(quev[:], ps_quev[:], AF.Copy)
            nc.gpsimd.tensor_copy(out=qurev[:, ::-1], in_=quev[:])

        # =========================================================
        # enc BiGRU -> V
        # =========================================================
        etraj = _bigru_picard(nc, tc, ap, "enc", lay["enc"], ident_bf,
                              [Pmat, quev], [Prev, qurev], TP, 0, KITS["enc"])
        Vmat, _ = _mats_from_traj(nc, ap, "V", etraj, TP, 1, rev=False)

        # V^T chunks [128(j), 128(d)]
        VT = [ap.tile([128, HID], BF16, name=f"VT{c}") for c in range(4)]
        with tc.tile_pool(name="vtps", bufs=2, space="PSUM") as vps:
            for c in range(4):
                pv = vps.tile([128, 128], BF16, tag="vt", name=f"vt{c}")
                nc.tensor.transpose(pv[:], Vmat[:, 128 * c:128 * (c + 1)], ident_bf[:])
                nc.scalar.activation(VT[c][:], pv[:], AF.Copy)

        # =========================================================
        # MultiFactor: Smax = max_f (V M_f V^T), A = softmax, MF = A V
        # =========================================================
        # S chunks paired into 2-bank tiles: one max op covers two chunks
        smpair = [ap.tile([128, 2 * TP], F32, name=f"smpair{p}") for p in range(2)]
        smax = [smpair[c // 2][:, TP * (c % 2):TP * (c % 2 + 1)] for c in range(4)]
        with tc.tile_pool(name="mfps", bufs=2, space="PSUM") as mp, \
             tc.tile_pool(name="mfu", bufs=2) as mu:
            for f in range(NF):
                ps_u = mp.tile([128, TP], F32, tag="u", name=f"u{f}")
                nc.tensor.matmul(ps_u[:], lhsT=ctxm[f][:], rhs=Vmat[:],
                                 start=True, stop=True)
                u_sb = mu.tile([128, TP], BF16, tag="usb", name=f"usb{f}")
                nc.scalar.activation(u_sb[:], ps_u[:], AF.Copy)
                for p in range(2):
                    ps_s = mp.tile([128, 2 * TP], F32, tag="s", name=f"s{f}_{p}")
                    for half in range(2):
                        c = 2 * p + half
                        nc.tensor.matmul(ps_s[:, TP * half:TP * (half + 1)],
                                         lhsT=u_sb[:, 128 * c:128 * (c + 1)], rhs=Vmat[:],
                                         start=True, stop=True)
                    if f == 0:
                        nc.scalar.activation(smpair[p][:], ps_s[:], AF.Copy)
                    else:
                        nc.vector.tensor_tensor(out=smpair[p][:], in0=ps_s[:],
                                                in1=smpair[p][:], op=OP.max)

        # softmax rows of smax -> A chunks
        Achunks = []
        for c in range(4):
            am = ap.tile([128, 1], F32, name=f"amx{c}")
            nc.vector.tensor_reduce(out=am[:], in_=smax[c][:], axis=AX.X, op=OP.max)
            amn = ap.tile([128, 1], F32, name=f"amn{c}")
            nc.vector.tensor_scalar(out=amn[:], in0=am[:], scalar1=-1.0, scalar2=None,
                                    op0=OP.mult)
            ae = ap.tile([128, TP], F32, name=f"ae{c}")
            nc.scalar.activation(ae[:], smax[c][:], AF.Exp, bias=amn[:])
            asum = ap.tile([128, 1], F32, name=f"asum{c}")
            nc.vector.tensor_reduce(out=asum[:], in_=ae[:], axis=AX.X, op=OP.add)
            arec = ap.tile([128, 1], F32, name=f"arec{c}")
            nc.vector.reciprocal(out=arec[:], in_=asum[:])
            a_c = ap.tile([128, TP], BF16, name=f"ac{c}")
            eng = nc.vector if c % 2 == 0 else nc.gpsimd
            eng.tensor_scalar(out=a_c[:], in0=ae[:], scalar1=arec[:], scalar2=None,
                              op0=OP.mult)
            Achunks.append(a_c)

        # A^T assembled per j-chunk: ATj[j] [128(j), 512(p)]
        ATj = [ap.tile([128, TP], BF16, name=f"atj{j}") for j in range(4)]
        with tc.tile_pool(name="atps", bufs=3, space="PSUM") as atp:
            for c in range(4):
                for j in range(4):
                    pt = atp.tile([128, 128], BF16, tag="at", name=f"at{c}_{j}")
                    nc.tensor.transpose(pt[:], Achunks[c][:, 128 * j:128 * (j + 1)], ident_bf[:])
                    if (c + j) % 2 == 0:
                        nc.vector.tensor_copy(out=ATj[j][:, 128 * c:128 * (c + 1)],
                                              in_=pt[:])
                    else:
                        nc.scalar.activation(ATj[j][:, 128 * c:128 * (c + 1)],
                                             pt[:], AF.Copy)

        # MF^T [128(d), 512(p)] then gated FF -> Y^T chunks
        MFT = ap.tile([128, TP], BF16, name="MFT")
        Ych, Yrev = [], []
        with tc.tile_pool(name="mftps", bufs=2, space="PSUM") as mfp:
            ps_mft = mfp.tile([128, TP], F32, name="ps_mft")
            for j in range(4):
                nc.tensor.matmul(ps_mft[:], lhsT=VT[j][:], rhs=ATj[j][:],
                                 start=(j == 0), stop=(j == 3))
            nc.scalar.activation(MFT[:], ps_mft[:], AF.Copy)

            for cc in range(2):
                ps_g = mfp.tile([128, TP], F32, tag="gff", name=f"gff{cc}")
                nc.tensor.matmul(ps_g[:], lhsT=wg[0][:, 128 * cc:128 * (cc + 1)], rhs=MFT[:],
                                 start=True, stop=False)
                nc.tensor.matmul(ps_g[:], lhsT=wg[1][:, 128 * cc:128 * (cc + 1)], rhs=Vmat[:],
                                 start=False, stop=True)
                sg = ap.tile([128, TP], BF16, name=f"sg{cc}")
                nc.scalar.activation(sg[:], ps_g[:], AF.Sigmoid, bias=bg[:, cc:cc + 1])
                y = ap.tile([128, TP], BF16, name=f"y{cc}")
                src = MFT if cc == 0 else Vmat
                nc.vector.tensor_tensor(out=y[:], in0=src[:], in1=sg[:], op=OP.mult)
                yr = ap.tile([128, TP], BF16, name=f"yr{cc}")
                nc.gpsimd.tensor_copy(out=yr[:, ::-1], in_=y[:])
                Ych.append(y)
                Yrev.append(yr)

        # =========================================================
        # begin / end BiGRUs, span logits
        # =========================================================
        btraj = _bigru_picard(nc, tc, ap, "begin", lay["begin"], ident_bf,
                              Ych, Yrev, TP, 0, KITS["begin"])
        Bmat, Brev = _mats_from_traj(nc, ap, "B", btraj, TP, 1)

        etraj2 = _bigru_picard(nc, tc, ap, "end", lay["end"], ident_bf,
                               [Bmat], [Brev], TP, 0, KITS["end"])
        Emat, _ = _mats_from_traj(nc, ap, "E", etraj2, TP, 1, rev=False)

        with tc.tile_pool(name="outps", bufs=1, space="PSUM") as op_:
            ps_sb = op_.tile([1, TP], F32, name="ps_sb")
            nc.tensor.matmul(ps_sb[:], lhsT=qrep[:], rhs=Bmat[:], start=True, stop=True)
            sb_sb = ap.tile([1, TP], F32, name="sb_sb")
            nc.scalar.activation(sb_sb[:], ps_sb[:], AF.Copy)
            ps_se = op_.tile([1, TP], F32, name="ps_se")
            nc.tensor.matmul(ps_se[:], lhsT=qrep[:], rhs=Emat[:], start=True, stop=True)
            se_sb = ap.tile([1, TP], F32, name="se_sb")
            nc.scalar.activation(se_sb[:], ps_se[:], AF.Copy)
            nc.sync.dma_start(out=out_d[0:1, :], in_=sb_sb[:])
            nc.sync.dma_start(out=out_d[1:2, :], in_=se_sb[:])

    return nc


# --------------------------------------------------------------------------
# host-side weight packing
# --------------------------------------------------------------------------

def _pack_layer(pf, pb):
    """Build gil [D,384], rec [128,384], brz [128,2], bn [128,2] from the
    (Wih, Whh, bih, bhh) tuples of both directions."""
    Wih_f, Whh_f, bih_f, bhh_f = [np.asarray(a, np.float32) for a in pf]
    Wih_b, Whh_b, bih_b, bhh_b = [np.asarray(a, np.float32) for a in pb]
    D = Wih_f.shape[1]
    gil = np.zeros((D, 384), np.float32)
    rec = np.zeros((128, 384), np.float32)
    for g in range(3):
        gil[:, 128 * g:128 * g + 64] = Wih_f[64 * g:64 * (g + 1), :].T
        gil[:, 128 * g + 64:128 * g + 128] = Wih_b[64 * g:64 * (g + 1), :].T
        rec[0:64, 128 * g:128 * g + 64] = Whh_f[64 * g:64 * (g + 1), :].T
        rec[64:128, 128 * g + 64:128 * g + 128] = Whh_b[64 * g:64 * (g + 1), :].T
    brz = np.zeros((128, 2), np.float32)
    bn = np.zeros((128, 2), np.float32)
    for g in range(2):
        brz[0:64, g] = bih_f[64 * g:64 * (g + 1)] + bhh_f[64 * g:64 * (g + 1)]
        brz[64:128, g] = bih_b[64 * g:64 * (g + 1)] + bhh_b[64 * g:64 * (g + 1)]
    bn[0:64, 0] = bih_f[128:192]
    bn[64:128, 0] = bih_b[128:192]
    bn[0:64, 1] = bhh_f[128:192]
    bn[64:128, 1] = bhh_b[128:192]
    bf = ml_dtypes.bfloat16
    return gil.astype(bf), rec.astype(bf), brz, bn


def _get_built():
    global _BUILT
    if _BUILT is None:
        _BUILT = _build_graph()
        _BUILT.finalize()
    return _BUILT


def _build_noop():
    """Minimal graph for calibrating the axon RPC floor."""
    nc = bacc.Bacc()
    i_d = nc.declare_dram_parameter("nin", [2, TP], F32, isOutput=False)
    o_d = nc.declare_dram_parameter("out", [2, TP], F32, isOutput=True)
    with tile.TileContext(nc) as tc, tc.tile_pool(name="p", bufs=1) as pp:
        t = pp.tile([2, TP], F32, name="t")
        nc.sync.dma_start(out=t[:], in_=i_d[:])
        nc.sync.dma_start(out=o_d[:], in_=t[:])
    return nc


class _Runner:
    """Persistent jit executor mirroring bass2jax.run_bass_via_pjrt's
    multi-core path, with resident inputs for repeat timing."""

    def __init__(self, nc):
        if not nc.is_finalized():
            nc.finalize()
        import jax
        from jax.experimental.shard_map import shard_map
        from jax.sharding import Mesh, PartitionSpec
        from concourse import bass2jax as b2j

        b2j.install_neuronx_cc_hook()
        self.nc = nc
        pname = nc.partition_id_tensor.name if nc.partition_id_tensor else None
        in_names, out_names, out_avals, zero_outs = [], [], [], []
        for alloc in nc.m.functions[0].allocations:
            if not isinstance(alloc, mybir.MemoryLocationSet):
                continue
            name = alloc.memorylocations[0].name
            if alloc.kind == "ExternalInput":
                if name != pname:
                    in_names.append(name)
            elif alloc.kind == "ExternalOutput":
                out_names.append(name)
                shape = tuple(alloc.tensor_shape)
                dtype = mybir.dt.np(alloc.dtype)
                out_avals.append(jax.core.ShapedArray(shape, dtype))
                zero_outs.append(np.zeros(shape, dtype))
        self.in_names, self.out_names = in_names, out_names
        self.out_avals, self.zero_outs = out_avals, zero_outs
        n_params, n_outs = len(in_names), len(out_names)
        all_names = in_names + out_names
        if pname is not None:
            all_names = all_names + [pname]

        def _body(*args):
            operands = list(args)
            if pname is not None:
                operands.append(b2j.partition_id_tensor())
            outs = b2j._bass_exec_p.bind(
                *operands, out_avals=tuple(out_avals), in_names=tuple(all_names),
                out_names=tuple(out_names), lowering_input_output_aliases=(),
                sim_require_finite=True, sim_require_nnan=True, nc=nc)
            return tuple(outs)

        devices = jax.devices()[:NCORES]
        self.mesh = Mesh(np.asarray(devices), ("core",))
        in_specs = (PartitionSpec("core"),) * (n_params + n_outs)
        out_specs = (PartitionSpec("core"),) * n_outs
        self.fn = jax.jit(
            shard_map(_body, mesh=self.mesh, in_specs=in_specs,
                      out_specs=out_specs, check_rep=False),
            donate_argnums=tuple(range(n_params, n_params + n_outs)),
            keep_unused=True)
        self._resident = None

    def set_inputs(self, in_maps):
        import jax
        from jax.sharding import NamedSharding, PartitionSpec

        sh = NamedSharding(self.mesh, PartitionSpec("core"))
        self._resident = [
            jax.device_put(
                np.concatenate([np.asarray(m[name]) for m in in_maps], axis=0), sh)
            for name in self.in_names
        ]

    def run(self):
        import jax
        from jax.sharding import NamedSharding, PartitionSpec

        sh = NamedSharding(self.mesh, PartitionSpec("core"))
        zeros = [jax.device_put(
            np.zeros((NCORES * z.shape[0], *z.shape[1:]), z.dtype), sh)
            for z in self.zero_outs]
        outs = self.fn(*self._resident, *zeros)
        outs = [np.asarray(o) for o in outs]
        return [
            {name: outs[i].reshape(NCORES, *self.out_avals[i].shape)[c]
             for i, name in enumerate(self.out_names)}
            for c in range(NCORES)
        ]

    def time_ns(self, iters=12):
        import time as _t
        best = float("inf")
        for _ in range(iters):
            t0 = _t.perf_counter()
            self.run()
            dt = _t.perf_counter() - t0
            best = min(best, dt)
        return best * 1e9


_RUNNER = None
_NOOP_NS = None


def kernel(passages, questions, params):
    global LAST_EXEC_NS, LAST_RESULTS, _RUNNER, _NOOP_NS
    passages = np.asarray(passages)
    questions = np.asarray(questions)
    p = dict(params)
    bf = ml_dtypes.bfloat16

    common = {
        "emb": np.asarray(p["emb"], np.float32),
        "wg": np.asarray(p["Wg"], np.float32).astype(bf),
        "wq": np.asarray(p["Wq"], np.float32).astype(bf),
        "bq": np.asarray(p["bq"], np.float32).reshape(128, 1),
        "ctxm": np.asarray(p["ctx_M"], np.float32).reshape(NF * HID, HID).astype(bf),
        "bg": np.asarray(p["bg"], np.float32).reshape(2, 128).T.copy(),
        "identb": np.eye(128, dtype=np.float32).astype(bf),
        "identf": np.eye(128, dtype=np.float32),
    }
    for lname in ("share", "enc", "begin", "end"):
        gil, rec, brz, bn = _pack_layer(p[f"{lname}_f"], p[f"{lname}_b"])
        common[f"{lname}_gil"] = gil
        common[f"{lname}_rec"] = rec
        common[f"{lname}_brz"] = brz
        common[f"{lname}_bn"] = bn

    in_maps = []
    for b_ in range(B):
        toks = np.concatenate([np.asarray(passages[b_], np.int64),
                               np.asarray(questions[b_], np.int64),
                               np.zeros(96, np.int64)])
        idx = toks.reshape(5, 128).T.astype(np.int32).copy()
        m = dict(common)
        m["idx"] = idx
        in_maps.append(m)

    nc = _get_built()
    if _RUNNER is None:
        _RUNNER = _Runner(nc)
    _RUNNER.set_inputs(in_maps)
    results = _RUNNER.run()
    LAST_RESULTS = results

    if bool(int(os.environ.get("BASS_PROFILE", "0"))):
        full_ns = _RUNNER.time_ns()
        if _NOOP_NS is None:
            nr = _Runner(_build_noop())
            nr.set_inputs([{"nin": np.zeros((2, TP), np.float32)} for _ in range(NCORES)])
            nr.run()
            _NOOP_NS = nr.time_ns()
        LAST_EXEC_NS = full_ns - _NOOP_NS
        print(f"[timing] full={full_ns/1e3:.1f}us noop={_NOOP_NS/1e3:.1f}us "
              f"-> kernel ~{LAST_EXEC_NS/1e3:.1f}us")

    loss = 0.0
    for b_ in range(B):
        o = np.asarray(results[b_]["out"], np.float64)
        sb, se = o[0], o[1]
        for v in (sb, se):
            mx = v.max()
            loss += TP * (mx + np.log(np.exp(v - mx).sum()))
        loss -= sb.sum() + se.sum()
    return np.float32(loss)
